# revision 3
# baseline (speedup 1.0000x reference)
"""Trainium2 Bass kernel for nn_ActionDecoder (MoE-routed 2-layer GELU MLP).

Problem: per batch row b (2048 rows x 16 timesteps), route through the
embodiment_ids[b]-th expert MLP: out = GELU(x @ W1[e] + b1[e]) @ W2[e] + b2[e].
x: [2048, 16, 512] f32, W1: [4, 512, 1024], W2: [4, 1024, 28].

Strategy (expert-parallel): host sorts batch rows by embodiment, gives each of
the 8 cores one expert (2 cores per expert, half the expert's rows each). Each
core runs a dense 2-layer MLP over its tokens with its own expert's weights
(weights are per-core *data*, so one SPMD program serves all cores). Activations
are fed transposed ([d, tok]) so both matmuls keep weights stationary; compute
in bf16 with fp32 PSUM accumulation.
"""

import numpy as np
import ml_dtypes

import concourse.bass as bass
import concourse.bacc as bacc
import concourse.mybir as mybir
from concourse.tile import TileContext
from concourse.bass_utils import run_bass_kernel_spmd

# Model dims (hardcoded per problem spec)
D = 512      # d_model
H = 1024     # hidden
A = 28       # max action dim
E = 4        # n embodiments
N_CORES = 8
P = 128      # partitions
TILE = 512   # tokens per matmul tile
KC = D // P  # 4 contraction chunks for layer 1
HC = H // P  # 8 hidden chunks

F32 = mybir.dt.float32
BF16 = mybir.dt.bfloat16

# Cache compiled program per token count (NTOK): (nc, ntok)
_PROGRAM_CACHE = {}

# Set by test harness to collect a profile: None | dict (filled with results)
TRACE_SINK = None


def _build_program(ntok):
    nt = ntok // TILE
    nc = bacc.Bacc()

    x_in = nc.declare_dram_parameter("x", [nt, P, KC, TILE], BF16, isOutput=False)
    w1_in = nc.declare_dram_parameter("w1", [P, KC, H], BF16, isOutput=False)
    w2_in = nc.declare_dram_parameter("w2", [P, HC, A], BF16, isOutput=False)
    b1_in = nc.declare_dram_parameter("b1", [P, HC], F32, isOutput=False)
    b2_in = nc.declare_dram_parameter("b2", [A, 1], F32, isOutput=False)
    out = nc.declare_dram_parameter("out", [A, ntok], F32, isOutput=True)

    with TileContext(nc) as tc:
        with (
            tc.tile_pool(name="wpool", bufs=1) as wpool,
            tc.tile_pool(name="xpool", bufs=3) as xpool,
            tc.tile_pool(name="hpool", bufs=2) as hpool,
            tc.tile_pool(name="opool", bufs=3) as opool,
            tc.tile_pool(name="ps_h", bufs=4, space="PSUM") as ps_h_pool,
            tc.tile_pool(name="ps_o", bufs=2, space="PSUM") as ps_o_pool,
        ):
            w1_sb = wpool.tile([P, KC, H], BF16)
            nc.sync.dma_start(out=w1_sb, in_=w1_in[:])
            w2_sb = wpool.tile([P, HC, A], BF16)
            nc.sync.dma_start(out=w2_sb, in_=w2_in[:])
            b1_sb = wpool.tile([P, HC], F32)
            nc.sync.dma_start(out=b1_sb, in_=b1_in[:])
            b2_sb = wpool.tile([A, 1], F32)
            nc.sync.dma_start(out=b2_sb, in_=b2_in[:])

            for t in range(nt):
                x_sb = xpool.tile([P, KC, TILE], BF16)
                nc.sync.dma_start(out=x_sb, in_=x_in[t])

                h_sb = hpool.tile([P, HC, TILE], BF16)
                for hc in range(HC):
                    ps = ps_h_pool.tile([P, TILE], F32)
                    for kc in range(KC):
                        nc.tensor.matmul(
                            ps,
                            w1_sb[:, kc, hc * P:(hc + 1) * P],
                            x_sb[:, kc],
                            start=(kc == 0),
                            stop=(kc == KC - 1),
                        )
                    # h = gelu(ps + b1) in bf16 (exact-erf Gelu table)
                    nc.scalar.activation(
                        h_sb[:, hc], ps,
                        mybir.ActivationFunctionType.Gelu,
                        bias=b1_sb[:, hc:hc + 1],
                    )

                o_ps = ps_o_pool.tile([A, TILE], F32)
                for hc in range(HC):
                    nc.tensor.matmul(
                        o_ps,
                        w2_sb[:, hc],
                        h_sb[:, hc],
                        start=(hc == 0),
                        stop=(hc == HC - 1),
                    )
                o_sb = opool.tile([A, TILE], F32)
                nc.vector.tensor_scalar_add(o_sb, o_ps, b2_sb)
                nc.sync.dma_start(out=out[:, t * TILE:(t + 1) * TILE], in_=o_sb)

    nc.finalize()
    return nc


def kernel(pred_action_latents, W1, b1, W2, b2, embodiment_ids):
    x = np.asarray(pred_action_latents)
    W1 = np.asarray(W1)
    b1 = np.asarray(b1)
    W2 = np.asarray(W2)
    b2 = np.asarray(b2)
    ids = np.asarray(embodiment_ids)

    B, T, _ = x.shape
    assert W1.shape[0] == E and N_CORES == 2 * E

    # --- Host-side routing/sharding ---
    order = np.argsort(ids, kind="stable")
    counts = np.bincount(ids, minlength=E)
    starts = np.concatenate([[0], np.cumsum(counts)])

    # core 2e, 2e+1 handle expert e (first/second half of its rows)
    core_rows = []
    for e in range(E):
        rows_e = order[starts[e]:starts[e + 1]]
        h1 = (len(rows_e) + 1) // 2
        core_rows.append(rows_e[:h1])
        core_rows.append(rows_e[h1:])

    max_tok = max(len(r) * T for r in core_rows)
    ntok = max(TILE, ((max_tok + TILE - 1) // TILE) * TILE)
    nt = ntok // TILE

    if ntok not in _PROGRAM_CACHE:
        _PROGRAM_CACHE[ntok] = _build_program(ntok)
    nc = _PROGRAM_CACHE[ntok]

    in_maps = []
    for c in range(N_CORES):
        e = c // 2
        rows = core_rows[c]
        ntok_real = len(rows) * T
        # tokens for this core: [ntok, D], zero-padded
        xr = np.zeros((ntok, D), dtype=np.float32)
        xr[:ntok_real] = x[rows].reshape(ntok_real, D)
        # device layout [nt, P, KC, TILE]: (t, p, kc, n) = xr[t*TILE+n, kc*P+p]
        x_dev = np.ascontiguousarray(
            xr.reshape(nt, TILE, KC, P).transpose(0, 3, 2, 1)
        ).astype(ml_dtypes.bfloat16)
        # [P, KC, H]: (p, kc, h) = W1[e, kc*P+p, h]
        w1_dev = np.ascontiguousarray(
            W1[e].reshape(KC, P, H).transpose(1, 0, 2)
        ).astype(ml_dtypes.bfloat16)
        # [P, HC, A]: (p, hc, a) = W2[e, hc*P+p, a]
        w2_dev = np.ascontiguousarray(
            W2[e].reshape(HC, P, A).transpose(1, 0, 2)
        ).astype(ml_dtypes.bfloat16)
        b1_dev = np.ascontiguousarray(b1[e].reshape(HC, P).T).astype(np.float32)
        b2_dev = np.ascontiguousarray(b2[e].reshape(A, 1)).astype(np.float32)
        in_maps.append({
            "x": x_dev, "w1": w1_dev, "w2": w2_dev, "b1": b1_dev, "b2": b2_dev,
        })

    trace = TRACE_SINK is not None
    res = run_bass_kernel_spmd(nc, in_maps, core_ids=list(range(N_CORES)),
                               trace=trace)
    if trace:
        TRACE_SINK["exec_time_ns"] = res.exec_time_ns
        TRACE_SINK["mean_exec_time_ns"] = res.mean_exec_time_ns
        TRACE_SINK["profile_json"] = res.profile_json

    # --- Host-side unshard ---
    out_full = np.zeros((B, T, A), dtype=np.float32)
    for c in range(N_CORES):
        rows = core_rows[c]
        if len(rows) == 0:
            continue
        o = np.asarray(res.results[c]["out"])  # [A, ntok] f32
        out_full[rows] = o[:, :len(rows) * T].T.reshape(len(rows), T, A)
    return out_full
